# revision 26
# baseline (speedup 1.0000x reference)
"""GCN mean-aggregation (DGL copy_src -> mean by dst) on 8 NeuronCores.

Strategy (dst-sharded, no collectives):
  - Host: edges are assigned to the core owning their dst row (core c owns
    rows [c*12500, (c+1)*12500)).  Within a core, dst nodes form 98 buckets
    of 128; src rows are split into 4 groups keyed by (src//50048, src&1) so
    gather indices fit int16 at a 256B descriptor stride while the table
    stays dense 128B rows (the parity picks the 128B half of each 256B row
    pair).  Edges are sorted by (bucket-wave, group, bucket, src) and each
    (bucket, group) run is padded to a static number of 128-edge tiles (max
    over the 8 cores), so a single program serves all cores.  Pad edges
    gather row 0 and are masked by a sentinel dst_local = 128.
    Per-node 1/max(indeg,1) is precomputed on the host.
  - Device (identical program per core), waves of 8 buckets largest-first:
      * per (wave, group), chunks of 8 tiles: batched dma_gather of bf16
        128B rows (elem 64 x bf16, stride 256B -- half the bytes of the
        256B-padded v1) into SBUF, round-robin over 4 SWDGE queues.
        HARD LIMIT: >1024 indices per dma_gather call crashes the exec
        unit (NRT_EXEC_UNIT_UNRECOVERABLE), independent of the descriptor
        carveout size; CoreSim does not model this.  Keep chunks at 8
        tiles (1024 idxs).
      * per wave: ALL one-hots in ONE DVE tensor_tensor is_equal with
        t-inner layout ohX[p, d*ntw + t] = (iota[d] == dstloc[p, t]).  Both
        operands are packed-inner (iotaRep is materialized d-major on the
        host), which qualifies for the DVE 2x_1p perf mode -- the old
        scalar_tensor_tensor broadcast form supported no perf mode at all
        (DVE busy 169us -> 82us).
      * per edge-tile: psum[:, :64] += ohX[:, d-slice]^T @ msgs  (bf16
        matmul, f32 acc, 8 psum tiles in flight; lhsT reads ohX with a
        t-strided AP)
      * per bucket: out = psum * recip on the Scalar engine (Copy w/ scale,
        bf16 out; host upcasts to f32)
  - Host: concatenate the 8 per-core [12500, 64] outputs.

Measured (trace runs): v1 (256B elems, STT one-hots) 383-454us; this
version 390-400us with gather HBM traffic halved and DVE freed.  The
wall in every variant is the Pool engine driving the SWDGE gather
stream at ~93% busy: ~1us fixed per call + ~1.5ns per descriptor
(generation/ring drain); ~150 calls x <=1024 descs.  Element size
(128B vs 256B) does NOT change the per-descriptor cost -- the DMA
moves a >=256B quantum either way.  Bigger calls would amortize the
fixed cost but 1536- and 2048-desc calls both crash the exec unit
(NRT_EXEC_UNIT_UNRECOVERABLE) regardless of carveout size, so 1024 is
a hard ceiling.  Deeper msgs/oh buffering (bufs 4/3) measured slower
(~410us), likely SBUF pressure; keep 3/2.
"""

import sys
from contextlib import ExitStack

import numpy as np
from ml_dtypes import bfloat16

sys.path.insert(0, "/opt/trn_rl_repo")

import concourse.bass as bass  # noqa: E402
import concourse.mybir as mybir  # noqa: E402
import concourse.tile as tile  # noqa: E402
from concourse import ap_utils  # noqa: E402
from concourse import bacc  # noqa: E402
from concourse.bass_utils import run_bass_kernel_spmd  # noqa: E402

N_NODES = 100000
N_EDGES = 1000000
D_FEAT = 64
N_CORES = 8
NODES_PER_CORE = N_NODES // N_CORES  # 12500
BUCKET = 128  # dst nodes per psum bucket (= one-hot width)
N_PAD = 100096  # nodes padded to a multiple of 256 (50048 row pairs)
HALF = N_PAD // 2  # 50048: nodes per range-group (2 ranges x 2 parities)
N_GROUPS = 4  # ELEM128: (src // HALF, src & 1); else src // GSZ
GSZ = N_PAD // N_GROUPS  # 25024 rows per range group (256B mode)
WAVE = 8  # buckets per wave
N_QUEUES = 4  # SWDGE queues (hw max)
ELEM128 = True  # 128B gather elements (raw descriptor path) vs 256B padded
CHUNK = 8  # tiles per dma_gather call; 1024 descs is a HARD ring limit
# (1536- and 2048-desc calls both crash the exec unit regardless of carveout)
DMA_SCRATCH = 16384  # descriptor carveout (default); bigger does NOT lift
# the 1024-desc per-call ring limit -- 2048-desc calls crash the exec unit
# (NRT_EXEC_UNIT_UNRECOVERABLE) even with a 4x carveout


def _dma_gather_raw(
    gp, out_ap, in_ap, idxs_ap, num_idxs, elem_size, elem_step, queue_num
):
    """BassGpSimd.dma_gather (non-transpose, DRAM source) with the 256B
    element-granularity assert relaxed: elements only need the descriptor
    STRIDE to be a 256B multiple; the transfer length can be 64B-aligned.
    Mirrors bass.py's lowering exactly otherwise."""
    gp._assert_queue_num(queue_num)
    assert idxs_ap.dtype == mybir.dt.int16
    assert in_ap.dtype == out_ap.dtype
    elem_size_bytes = elem_size * mybir.dt.size(in_ap.dtype)
    assert elem_size_bytes > 0 and elem_size_bytes % 64 == 0
    assert in_ap.space == bass.MemorySpace.DRAM
    assert idxs_ap.space == bass.MemorySpace.SBUF
    assert out_ap.space == bass.MemorySpace.SBUF
    assert ap_utils.ap_is_contiguous(in_ap.ap[1:])
    assert ap_utils.ap_is_contiguous(out_ap.ap[1:])
    assert ap_utils.ap_is_contiguous(idxs_ap.ap[1:])
    assert in_ap.ap[-1][1] == out_ap.ap[-1][1] == elem_size
    assert out_ap.ap[0][1] * out_ap.ap[1][1] == num_idxs
    assert in_ap.ap[0][0] == elem_step
    stride_bytes = elem_step * mybir.dt.size(in_ap.dtype)
    stride_bytes_256, rem = divmod(stride_bytes, 256)
    assert rem == 0 and stride_bytes_256 < 256
    _in_ap = gp.lower_ap_dma(in_ap, for_custom_bir_dma=True)
    _idxs_ap = gp.lower_ap(idxs_ap)
    _out_ap = gp.lower_ap(out_ap)
    return gp.add_instruction(
        mybir.InstDMAGatherAnt(
            name=gp.bass.get_next_instruction_name(),
            ins=[*_in_ap, _idxs_ap, gp.lower_val_access(gp.to_reg(num_idxs))],
            outs=[_out_ap],
            transpose=False,
            num_idxs=num_idxs,
            elem_size=elem_size,
            stride_bytes_256=stride_bytes_256,
            gen_mode=0,
            single_packet=True,
            queue_num=queue_num,
            sbuf_tokens_per_rank=0,
            sbuf_free_dim_per_rank=0,
            sbuf_free_dim_pad_per_rank=0,
            sbuf_byte_offset=0,
        )
    )


def _schedule(cnt_max, nb, ngroups):
    """Static schedule from per-(bucket, group) max edge counts.

    cnt_max: [nb, ngroups] max edge count over cores.
    Returns tiles-per-region, waves, per-call and per-wave info.
    """
    tbg = -(-cnt_max // 128)  # [nb, ngroups]
    for b in range(nb):
        if tbg[b].sum() == 0:
            tbg[b, 0] = 1  # ensure psum gets reset even for empty buckets

    waves = [range(w, min(w + WAVE, nb)) for w in range(0, nb, WAVE)]
    # region order: (wave, group, bucket-in-wave)
    region_tile0 = np.zeros((nb, ngroups), np.int64)
    calls = []  # [wave][group] -> (tile0, ntiles)
    t = 0
    for wv in waves:
        wcalls = []
        for g in range(ngroups):
            c0 = t
            for b in wv:
                region_tile0[b, g] = t
                t += int(tbg[b, g])
            wcalls.append((c0, t - c0))
        calls.append(wcalls)
    return {
        "tbg": tbg,
        "waves": waves,
        "region_tile0": region_tile0,
        "calls": calls,
        "nt": t,
        "ntw": [sum(n for (_, n) in c) for c in calls],
        "wave_t0": [c[0][0] for c in calls],
    }


def _prep(src, dst):
    """Sort/group/pad edges; build per-core device inputs + static schedule."""
    npc, bucket, ngroups = NODES_PER_CORE, BUCKET, N_GROUPS
    src = np.asarray(src, dtype=np.int64)
    dst = np.asarray(dst, dtype=np.int64)
    nb = -(-npc // bucket)
    nw = -(-nb // WAVE)

    core = dst // npc
    b = (dst - core * npc) // bucket
    if ELEM128:
        g = (src // HALF) * 2 + (src & 1)  # (range, parity)
    else:
        g = src // GSZ
    w = b // WAVE

    cnt = np.zeros((N_CORES, nb, ngroups), np.int64)
    np.add.at(cnt, (core, b, g), 1)
    sched = _schedule(cnt.max(axis=0), nb, ngroups)
    tbg, region_tile0, nt = sched["tbg"], sched["region_tile0"], sched["nt"]
    nslot = nt * 128

    # global sort by (core, wave, group, bucket, src)
    key = ((core * nw + w) * ngroups + g) * nb + b
    order = np.lexsort((src, key))
    ss, ks = src[order], key[order]
    dl = (dst - (core * npc + b * bucket))[order]  # dst_local in [0, bucket)

    kcnt = np.bincount(ks, minlength=N_CORES * nw * ngroups * nb)
    kstart = np.zeros(kcnt.shape[0] + 1, np.int64)
    np.cumsum(kcnt, out=kstart[1:])
    rank = np.arange(ss.shape[0], dtype=np.int64) - kstart[ks]

    slot_base = region_tile0 * 128  # [nb, ngroups], within-core slot offset
    pos = core[order] * nslot + slot_base[b[order], g[order]] + rank

    # per-slot local gather index (fits int16)
    idx_slot = np.zeros(N_CORES * nslot, np.int64)  # pad: row 0 of group
    dstloc = np.full(N_CORES * nslot, float(bucket), np.float32)
    idx_slot[pos] = ((ss % HALF) >> 1) if ELEM128 else (ss % GSZ)
    dstloc[pos] = dl.astype(np.float32)

    idx16 = idx_slot.astype(np.int16)
    # wrapped index layout: idx j -> partition j%16, col j//16 (x8 replicas)
    idx16 = idx16.reshape(N_CORES, nt * 8, 16)
    idxtab = np.ascontiguousarray(idx16.transpose(0, 2, 1))  # [C, 16, nt*8]
    idxtab = np.tile(idxtab, (1, 8, 1))  # [C, 128, nt*8]

    # dst-local table [C, 128, nt]: partition = slot-in-tile, col = tile
    dst_t = np.ascontiguousarray(
        dstloc.reshape(N_CORES, nt, 128).transpose(0, 2, 1)
    ).astype(bfloat16)

    # per-node 1/max(indegree, 1), laid out [C, 128, nb]: node b*128+p -> (p, b)
    counts = np.bincount(dst, minlength=N_NODES).astype(np.float32)
    rec = 1.0 / np.maximum(counts, 1.0)
    rec_t = np.ones((N_CORES, nb * 128), np.float32)
    rec_t[:, :npc] = rec.reshape(N_CORES, npc)
    rec_t = np.ascontiguousarray(
        rec_t.reshape(N_CORES, nb, 128).transpose(0, 2, 1)
    )  # [C, 128, nb]

    # iotaRep [128, 128 * ntw_max], d-major with t inner: value(d, t) = d
    ntw_max = max(sched["ntw"])
    iota_rep = np.broadcast_to(
        np.repeat(np.arange(BUCKET, dtype=np.float32), ntw_max).astype(bfloat16),
        (128, BUCKET * ntw_max),
    )
    sched["ntw_max"] = ntw_max
    return idxtab, dst_t, rec_t, np.ascontiguousarray(iota_rep), sched


def _build(sched):
    """Build the (per-core) Bass program."""
    npc = NODES_PER_CORE
    nb = -(-npc // BUCKET)
    nt = sched["nt"]
    tbg, region_tile0 = sched["tbg"], sched["region_tile0"]
    ntw_max = sched["ntw_max"]
    ntl_max = max(n for c in sched["calls"] for (_, n) in c)
    f32 = mybir.dt.float32
    bf16 = mybir.dt.bfloat16
    i16 = mybir.dt.int16

    nc = bacc.Bacc(
        "TRN2",
        target_bir_lowering=False,
        debug=False,
        num_swdge_queues=N_QUEUES,
        dynamic_dma_scratch_size=DMA_SCRATCH,
    )
    # ELEM128: dense 128B rows viewed as 50048 x 256B row pairs (256B stride);
    # else: 256B rows (64 real + 64 pad cols)
    emb = nc.dram_tensor(
        "emb",
        [HALF, 2 * D_FEAT] if ELEM128 else [N_PAD, 2 * D_FEAT],
        bf16,
        kind="ExternalInput",
    )
    idx_t = nc.dram_tensor("idx_t", [128, nt * 8], i16, kind="ExternalInput")
    dst_t = nc.dram_tensor("dst_t", [128, nt], bf16, kind="ExternalInput")
    rec_t = nc.dram_tensor("rec_t", [128, nb], f32, kind="ExternalInput")
    iota_t = nc.dram_tensor(
        "iota_t", [128, BUCKET * ntw_max], bf16, kind="ExternalInput"
    )
    # bf16 output halves the write traffic; host upcasts to f32
    out = nc.dram_tensor("out", [npc, D_FEAT], bf16, kind="ExternalOutput")

    # process waves largest-first so the pipeline tail is short
    nwv = len(sched["waves"])
    wave_order = sorted(range(nwv), key=lambda w: -sched["ntw"][w])

    with tile.TileContext(nc) as tc, ExitStack() as ctx:
        const_p = ctx.enter_context(tc.tile_pool(name="const", bufs=1))
        idx_p = ctx.enter_context(tc.tile_pool(name="idx", bufs=1))
        msgs_p = ctx.enter_context(tc.tile_pool(name="msgs", bufs=3))
        oh_p = ctx.enter_context(tc.tile_pool(name="oh", bufs=2))
        ps_p = ctx.enter_context(tc.tile_pool(name="ps", bufs=8, space="PSUM"))
        outp_p = ctx.enter_context(tc.tile_pool(name="outp", bufs=3))

        iota_rep = const_p.tile([128, BUCKET * ntw_max], bf16)
        nc.sync.dma_start(out=iota_rep[:], in_=iota_t[:, :])
        recall = const_p.tile([128, nb], f32)
        nc.sync.dma_start(out=recall[:], in_=rec_t[:, :])

        qn = 0
        for wvi in wave_order:
            wv = sched["waves"][wvi]
            t0w, ntw = sched["wave_t0"][wvi], sched["ntw"][wvi]
            # idx + dstloc slices for this wave
            idxg = {}
            for gg in range(N_GROUPS):
                t0, ntl = sched["calls"][wvi][gg]
                if ntl == 0:
                    continue
                ig = idx_p.tile([128, ntl * 8], i16, tag=f"idx{wvi}g{gg}")
                nc.sync.dma_start(
                    out=ig[:], in_=idx_t[:, t0 * 8 : (t0 + ntl) * 8]
                )
                idxg[gg] = ig
            dstw = idx_p.tile([128, ntw], bf16, tag=f"dst{wvi}")
            nc.sync.dma_start(out=dstw[:], in_=dst_t[:, t0w : t0w + ntw])

            # gathers chunked at CHUNK tiles (1024 idxs) per call: bigger
            # calls (2048+) crash the exec unit (NRT_EXEC_UNIT_UNRECOVERABLE)
            # -- a HW SWDGE limit CoreSim does not model.  ELEM128 gathers
            # 64 x bf16 = 128B at a 256B stride (group = range half x
            # parity); 256B mode gathers full padded rows.
            mrow = D_FEAT if ELEM128 else 2 * D_FEAT
            msgs = {}
            for gg in range(N_GROUPS):
                t0, ntl = sched["calls"][wvi][gg]
                if ntl == 0:
                    continue
                for sc in range(0, ntl, CHUNK):
                    k = min(CHUNK, ntl - sc)
                    m = msgs_p.tile(
                        [128, CHUNK * mrow], bf16, tag=f"msgs{gg}c{sc // CHUNK}"
                    )
                    msgs[(gg, sc // CHUNK)] = m
                    if ELEM128:
                        r, p = gg >> 1, gg & 1
                        _dma_gather_raw(
                            nc.gpsimd,
                            out_ap=m[:, : k * mrow].rearrange(
                                "p (t e) -> p t e", e=mrow
                            ),
                            in_ap=emb[
                                r * (HALF // 2) : (r + 1) * (HALF // 2),
                                p * D_FEAT : (p + 1) * D_FEAT,
                            ],
                            idxs_ap=idxg[gg][:, sc * 8 : (sc + k) * 8],
                            num_idxs=k * 128,
                            elem_size=D_FEAT,
                            elem_step=2 * D_FEAT,
                            queue_num=qn,
                        )
                    else:
                        nc.gpsimd.dma_gather(
                            out_ap=m[:, : k * mrow].rearrange(
                                "p (t e) -> p t e", e=mrow
                            ),
                            in_ap=emb[gg * GSZ : (gg + 1) * GSZ, :],
                            idxs_ap=idxg[gg][:, sc * 8 : (sc + k) * 8],
                            num_idxs=k * 128,
                            num_idxs_reg=k * 128,
                            elem_size=mrow,
                            queue_num=qn,
                        )
                    qn = (qn + 1) % N_QUEUES

            # ALL one-hots of the wave in one DVE op (2x_1p eligible):
            # ohX[p, d*ntw + t] = (iota[d] == dstloc[p, t])
            ohx = oh_p.tile([128, BUCKET * ntw_max], bf16)
            ohv = ohx[:, : BUCKET * ntw].rearrange("p (d t) -> p d t", t=ntw)
            nc.vector.tensor_tensor(
                out=ohv,
                in0=iota_rep[:].rearrange("p (d t) -> p d t", t=ntw_max)[
                    :, :, :ntw
                ],
                in1=dstw[:].unsqueeze(1).broadcast_to([128, BUCKET, ntw]),
                op=mybir.AluOpType.is_equal,
            )

            for bb in wv:
                passes = [
                    (gg, region_tile0[bb, gg] + j)
                    for gg in range(N_GROUPS)
                    for j in range(int(tbg[bb, gg]))
                ]
                psum = ps_p.tile([BUCKET, D_FEAT], f32)
                for i, (gg, t) in enumerate(passes):
                    tloc = int(t) - t0w
                    rel = int(t) - sched["calls"][wvi][gg][0]
                    off = (rel % CHUNK) * mrow
                    nc.tensor.matmul(
                        out=psum[:],
                        lhsT=ohv[:, :, tloc],
                        rhs=msgs[(gg, rel // CHUNK)][:, off : off + D_FEAT],
                        start=(i == 0),
                        stop=(i == len(passes) - 1),
                    )
                nrows = min(BUCKET, npc - bb * BUCKET)
                ot = outp_p.tile([BUCKET, D_FEAT], bf16)
                nc.scalar.activation(
                    out=ot[:],
                    in_=psum[:],
                    func=mybir.ActivationFunctionType.Copy,
                    scale=recall[:, bb : bb + 1],
                )
                nc.sync.dma_start(
                    out=out[bb * BUCKET : bb * BUCKET + nrows, :],
                    in_=ot[:nrows, :],
                )

    nc.compile()
    return nc


_CACHE = {}


def _run(embeddings, src, dst, trace=False, trace_kwargs=None):
    embeddings = np.asarray(embeddings, dtype=np.float32)
    if ELEM128:
        embp = np.zeros((N_PAD, D_FEAT), bfloat16)
        embp[:N_NODES] = embeddings.astype(bfloat16)
        embp = embp.reshape(HALF, 2 * D_FEAT)
    else:
        embp = np.zeros((N_PAD, 2 * D_FEAT), bfloat16)
        embp[:N_NODES, :D_FEAT] = embeddings.astype(bfloat16)
    idxtab, dst_t, rec_t, iota_rep, sched = _prep(src, dst)
    key = sched["tbg"].tobytes()
    if key not in _CACHE:
        _CACHE[key] = _build(sched)
    nc = _CACHE[key]

    in_maps = [
        {
            "emb": embp,
            "idx_t": idxtab[c],
            "dst_t": dst_t[c],
            "rec_t": rec_t[c],
            "iota_t": iota_rep,
        }
        for c in range(N_CORES)
    ]
    res = run_bass_kernel_spmd(
        nc,
        in_maps,
        core_ids=list(range(N_CORES)),
        trace=trace,
        **(trace_kwargs or {}),
    )
    out = np.concatenate(
        [res.results[c]["out"] for c in range(N_CORES)], axis=0
    ).astype(np.float32)
    return out, res


def kernel(embeddings, src, dst):
    out, _ = _run(embeddings, src, dst, trace=False)
    return out


# revision 29
# speedup vs baseline: 1.0003x; 1.0003x over previous
"""GCN mean-aggregation (DGL copy_src -> mean by dst) on 8 NeuronCores.

Strategy (dst-sharded, no collectives):
  - Host: edges are assigned to the core owning their dst row (core c owns
    rows [c*12500, (c+1)*12500)).  Within a core, dst nodes form 98 buckets
    of 128; src rows are split into 4 groups keyed by (src//50048, src&1) so
    gather indices fit int16 at a 256B descriptor stride while the table
    stays dense 128B rows (the parity picks the 128B half of each 256B row
    pair).  Edges are sorted by (bucket-wave, group, bucket, src) and each
    (bucket, group) run is padded to a static number of 128-edge tiles (max
    over the 8 cores), so a single program serves all cores.  Pad edges
    gather row 0 and are masked by a sentinel dst_local = 128.
    Per-node 1/max(indeg,1) is precomputed on the host.
  - Device (identical program per core), waves of 8 buckets largest-first:
      * per (wave, group), chunks of 8 tiles: batched dma_gather of bf16
        128B rows (elem 64 x bf16, stride 256B -- half the bytes of the
        256B-padded v1) into SBUF, round-robin over 4 SWDGE queues.
        HARD LIMIT: >1024 indices per dma_gather call crashes the exec
        unit (NRT_EXEC_UNIT_UNRECOVERABLE), independent of the descriptor
        carveout size; CoreSim does not model this.  Keep chunks at 8
        tiles (1024 idxs).
      * per wave: ALL one-hots in ONE DVE tensor_tensor is_equal with
        t-inner layout ohX[p, d*ntw + t] = (iota[d] == dstloc[p, t]).  Both
        operands are packed-inner (iotaRep is materialized d-major on the
        host), which qualifies for the DVE 2x_1p perf mode -- the old
        scalar_tensor_tensor broadcast form supported no perf mode at all
        (DVE busy 169us -> 82us).
      * per edge-tile: psum[:, :64] += ohX[:, d-slice]^T @ msgs  (bf16
        matmul, f32 acc, 8 psum tiles in flight; lhsT reads ohX with a
        t-strided AP)
      * per bucket: out = psum * recip on the Scalar engine (Copy w/ scale,
        bf16 out; host upcasts to f32)
  - Host: concatenate the 8 per-core [12500, 64] outputs.

Measured (trace runs): v1 (256B elems, STT one-hots) 383-454us; this
version 390-400us with gather HBM traffic halved and DVE freed.  The
wall in every variant is the Pool engine driving the SWDGE gather
stream at ~93% busy: ~1us fixed per call + ~1.5ns per descriptor
(generation/ring drain); ~150 calls x <=1024 descs.  Element size
(128B vs 256B) does NOT change the per-descriptor cost -- the DMA
moves a >=256B quantum either way.  Bigger calls would amortize the
fixed cost but 1536- and 2048-desc calls both crash the exec unit
(NRT_EXEC_UNIT_UNRECOVERABLE) regardless of carveout size, so 1024 is
a hard ceiling.  Deeper msgs/oh buffering (bufs 4/3) measured slower
(~410us), likely SBUF pressure; keep 3/2.
"""

import sys
from contextlib import ExitStack

import numpy as np
from ml_dtypes import bfloat16

sys.path.insert(0, "/opt/trn_rl_repo")

import concourse.bass as bass  # noqa: E402
import concourse.mybir as mybir  # noqa: E402
import concourse.tile as tile  # noqa: E402
from concourse import ap_utils  # noqa: E402
from concourse import bacc  # noqa: E402
from concourse.bass_utils import run_bass_kernel_spmd  # noqa: E402

N_NODES = 100000
N_EDGES = 1000000
D_FEAT = 64
N_CORES = 8
NODES_PER_CORE = N_NODES // N_CORES  # 12500
BUCKET = 128  # dst nodes per psum bucket (= one-hot width)
N_PAD = 100096  # nodes padded to a multiple of 256 (50048 row pairs)
HALF = N_PAD // 2  # 50048: nodes per range-group (2 ranges x 2 parities)
N_GROUPS = 4  # ELEM128: (src // HALF, src & 1); else src // GSZ
GSZ = N_PAD // N_GROUPS  # 25024 rows per range group (256B mode)
WAVE = 8  # buckets per wave
N_QUEUES = 4  # SWDGE queues (hw max)
ELEM128 = True  # 128B gather elements (raw descriptor path) vs 256B padded
CHUNK = 8  # tiles per dma_gather call; 1024 descs is a HARD ring limit
# (1536- and 2048-desc calls both crash the exec unit regardless of carveout)
DMA_SCRATCH = 16384  # descriptor carveout (default); bigger does NOT lift
# the 1024-desc per-call ring limit -- 2048-desc calls crash the exec unit
# (NRT_EXEC_UNIT_UNRECOVERABLE) even with a 4x carveout


def _dma_gather_raw(
    gp, out_ap, in_ap, idxs_ap, num_idxs, elem_size, elem_step, queue_num
):
    """BassGpSimd.dma_gather (non-transpose, DRAM source) with the 256B
    element-granularity assert relaxed: elements only need the descriptor
    STRIDE to be a 256B multiple; the transfer length can be 64B-aligned.
    Mirrors bass.py's lowering exactly otherwise."""
    gp._assert_queue_num(queue_num)
    assert idxs_ap.dtype == mybir.dt.int16
    assert in_ap.dtype == out_ap.dtype
    elem_size_bytes = elem_size * mybir.dt.size(in_ap.dtype)
    assert elem_size_bytes > 0 and elem_size_bytes % 64 == 0
    assert in_ap.space == bass.MemorySpace.DRAM
    assert idxs_ap.space == bass.MemorySpace.SBUF
    assert out_ap.space == bass.MemorySpace.SBUF
    assert ap_utils.ap_is_contiguous(in_ap.ap[1:])
    assert ap_utils.ap_is_contiguous(out_ap.ap[1:])
    assert ap_utils.ap_is_contiguous(idxs_ap.ap[1:])
    assert in_ap.ap[-1][1] == out_ap.ap[-1][1] == elem_size
    assert out_ap.ap[0][1] * out_ap.ap[1][1] == num_idxs
    assert in_ap.ap[0][0] == elem_step
    stride_bytes = elem_step * mybir.dt.size(in_ap.dtype)
    stride_bytes_256, rem = divmod(stride_bytes, 256)
    assert rem == 0 and stride_bytes_256 < 256
    _in_ap = gp.lower_ap_dma(in_ap, for_custom_bir_dma=True)
    _idxs_ap = gp.lower_ap(idxs_ap)
    _out_ap = gp.lower_ap(out_ap)
    return gp.add_instruction(
        mybir.InstDMAGatherAnt(
            name=gp.bass.get_next_instruction_name(),
            ins=[*_in_ap, _idxs_ap, gp.lower_val_access(gp.to_reg(num_idxs))],
            outs=[_out_ap],
            transpose=False,
            num_idxs=num_idxs,
            elem_size=elem_size,
            stride_bytes_256=stride_bytes_256,
            gen_mode=0,
            single_packet=True,
            queue_num=queue_num,
            sbuf_tokens_per_rank=0,
            sbuf_free_dim_per_rank=0,
            sbuf_free_dim_pad_per_rank=0,
            sbuf_byte_offset=0,
        )
    )


def _schedule(cnt_max, nb, ngroups):
    """Static schedule from per-(bucket, group) max edge counts.

    cnt_max: [nb, ngroups] max edge count over cores.
    Returns tiles-per-region, waves, per-call and per-wave info.
    """
    tbg = -(-cnt_max // 128)  # [nb, ngroups]
    for b in range(nb):
        if tbg[b].sum() == 0:
            tbg[b, 0] = 1  # ensure psum gets reset even for empty buckets

    waves = [range(w, min(w + WAVE, nb)) for w in range(0, nb, WAVE)]
    # region order: (wave, group, bucket-in-wave)
    region_tile0 = np.zeros((nb, ngroups), np.int64)
    calls = []  # [wave][group] -> (tile0, ntiles)
    t = 0
    for wv in waves:
        wcalls = []
        for g in range(ngroups):
            c0 = t
            for b in wv:
                region_tile0[b, g] = t
                t += int(tbg[b, g])
            wcalls.append((c0, t - c0))
        calls.append(wcalls)
    return {
        "tbg": tbg,
        "waves": waves,
        "region_tile0": region_tile0,
        "calls": calls,
        "nt": t,
        "ntw": [sum(n for (_, n) in c) for c in calls],
        "wave_t0": [c[0][0] for c in calls],
    }


def _prep(src, dst):
    """Sort/group/pad edges; build per-core device inputs + static schedule."""
    npc, bucket, ngroups = NODES_PER_CORE, BUCKET, N_GROUPS
    src = np.asarray(src, dtype=np.int64)
    dst = np.asarray(dst, dtype=np.int64)
    nb = -(-npc // bucket)
    nw = -(-nb // WAVE)

    core = dst // npc
    b = (dst - core * npc) // bucket
    if ELEM128:
        g = (src // HALF) * 2 + (src & 1)  # (range, parity)
    else:
        g = src // GSZ
    w = b // WAVE

    cnt = np.zeros((N_CORES, nb, ngroups), np.int64)
    np.add.at(cnt, (core, b, g), 1)
    sched = _schedule(cnt.max(axis=0), nb, ngroups)
    tbg, region_tile0, nt = sched["tbg"], sched["region_tile0"], sched["nt"]
    nslot = nt * 128

    # global sort by (core, wave, group, bucket, src)
    key = ((core * nw + w) * ngroups + g) * nb + b
    order = np.lexsort((src, key))
    ss, ks = src[order], key[order]
    dl = (dst - (core * npc + b * bucket))[order]  # dst_local in [0, bucket)

    kcnt = np.bincount(ks, minlength=N_CORES * nw * ngroups * nb)
    kstart = np.zeros(kcnt.shape[0] + 1, np.int64)
    np.cumsum(kcnt, out=kstart[1:])
    rank = np.arange(ss.shape[0], dtype=np.int64) - kstart[ks]

    slot_base = region_tile0 * 128  # [nb, ngroups], within-core slot offset
    pos = core[order] * nslot + slot_base[b[order], g[order]] + rank

    # per-slot local gather index (fits int16)
    idx_slot = np.zeros(N_CORES * nslot, np.int64)  # pad: row 0 of group
    dstloc = np.full(N_CORES * nslot, float(bucket), np.float32)
    idx_slot[pos] = ((ss % HALF) >> 1) if ELEM128 else (ss % GSZ)
    dstloc[pos] = dl.astype(np.float32)

    idx16 = idx_slot.astype(np.int16)
    # wrapped index layout: idx j -> partition j%16, col j//16 (x8 replicas)
    idx16 = idx16.reshape(N_CORES, nt * 8, 16)
    idxtab = np.ascontiguousarray(idx16.transpose(0, 2, 1))  # [C, 16, nt*8]
    idxtab = np.tile(idxtab, (1, 8, 1))  # [C, 128, nt*8]

    # dst-local table [C, 128, nt]: partition = slot-in-tile, col = tile
    dst_t = np.ascontiguousarray(
        dstloc.reshape(N_CORES, nt, 128).transpose(0, 2, 1)
    ).astype(bfloat16)

    # per-node 1/max(indegree, 1), laid out [C, 128, nb]: node b*128+p -> (p, b)
    counts = np.bincount(dst, minlength=N_NODES).astype(np.float32)
    rec = 1.0 / np.maximum(counts, 1.0)
    rec_t = np.ones((N_CORES, nb * 128), np.float32)
    rec_t[:, :npc] = rec.reshape(N_CORES, npc)
    rec_t = np.ascontiguousarray(
        rec_t.reshape(N_CORES, nb, 128).transpose(0, 2, 1)
    )  # [C, 128, nb]

    # iotaRep [128, 128 * ntw_max], d-major with t inner: value(d, t) = d
    ntw_max = max(sched["ntw"])
    iota_rep = np.broadcast_to(
        np.repeat(np.arange(BUCKET, dtype=np.float32), ntw_max).astype(bfloat16),
        (128, BUCKET * ntw_max),
    )
    sched["ntw_max"] = ntw_max
    return idxtab, dst_t, rec_t, np.ascontiguousarray(iota_rep), sched


def _build(sched):
    """Build the (per-core) Bass program."""
    npc = NODES_PER_CORE
    nb = -(-npc // BUCKET)
    nt = sched["nt"]
    tbg, region_tile0 = sched["tbg"], sched["region_tile0"]
    ntw_max = sched["ntw_max"]
    ntl_max = max(n for c in sched["calls"] for (_, n) in c)
    f32 = mybir.dt.float32
    bf16 = mybir.dt.bfloat16
    i16 = mybir.dt.int16

    nc = bacc.Bacc(
        "TRN2",
        target_bir_lowering=False,
        debug=False,
        num_swdge_queues=N_QUEUES,
        dynamic_dma_scratch_size=DMA_SCRATCH,
    )
    # ELEM128: dense 128B rows viewed as 50048 x 256B row pairs (256B stride);
    # else: 256B rows (64 real + 64 pad cols)
    emb = nc.dram_tensor(
        "emb",
        [HALF, 2 * D_FEAT] if ELEM128 else [N_PAD, 2 * D_FEAT],
        bf16,
        kind="ExternalInput",
    )
    idx_t = nc.dram_tensor("idx_t", [128, nt * 8], i16, kind="ExternalInput")
    dst_t = nc.dram_tensor("dst_t", [128, nt], bf16, kind="ExternalInput")
    rec_t = nc.dram_tensor("rec_t", [128, nb], f32, kind="ExternalInput")
    iota_t = nc.dram_tensor(
        "iota_t", [128, BUCKET * ntw_max], bf16, kind="ExternalInput"
    )
    # bf16 output halves the write traffic; host upcasts to f32
    out = nc.dram_tensor("out", [npc, D_FEAT], bf16, kind="ExternalOutput")

    # process waves largest-first so the pipeline tail is short
    nwv = len(sched["waves"])
    wave_order = sorted(range(nwv), key=lambda w: -sched["ntw"][w])

    with tile.TileContext(nc) as tc, ExitStack() as ctx:
        const_p = ctx.enter_context(tc.tile_pool(name="const", bufs=1))
        idx_p = ctx.enter_context(tc.tile_pool(name="idx", bufs=1))
        msgs_p = ctx.enter_context(tc.tile_pool(name="msgs", bufs=3))
        oh_p = ctx.enter_context(tc.tile_pool(name="oh", bufs=2))
        ps_p = ctx.enter_context(tc.tile_pool(name="ps", bufs=8, space="PSUM"))
        outp_p = ctx.enter_context(tc.tile_pool(name="outp", bufs=3))

        iota_rep = const_p.tile([128, BUCKET * ntw_max], bf16)
        nc.sync.dma_start(out=iota_rep[:], in_=iota_t[:, :])
        recall = const_p.tile([128, nb], f32)
        nc.sync.dma_start(out=recall[:], in_=rec_t[:, :])

        qn = 0
        for wvi in wave_order:
            wv = sched["waves"][wvi]
            t0w, ntw = sched["wave_t0"][wvi], sched["ntw"][wvi]
            # idx + dstloc slices for this wave
            idxg = {}
            for gg in range(N_GROUPS):
                t0, ntl = sched["calls"][wvi][gg]
                if ntl == 0:
                    continue
                ig = idx_p.tile([128, ntl * 8], i16, tag=f"idx{wvi}g{gg}")
                nc.sync.dma_start(
                    out=ig[:], in_=idx_t[:, t0 * 8 : (t0 + ntl) * 8]
                )
                idxg[gg] = ig
            dstw = idx_p.tile([128, ntw], bf16, tag=f"dst{wvi}")
            nc.sync.dma_start(out=dstw[:], in_=dst_t[:, t0w : t0w + ntw])

            # gathers chunked at CHUNK tiles (1024 idxs) per call: bigger
            # calls (2048+) crash the exec unit (NRT_EXEC_UNIT_UNRECOVERABLE)
            # -- a HW SWDGE limit CoreSim does not model.  ELEM128 gathers
            # 64 x bf16 = 128B at a 256B stride (group = range half x
            # parity); 256B mode gathers full padded rows.
            mrow = D_FEAT if ELEM128 else 2 * D_FEAT
            msgs = {}
            for gg in range(N_GROUPS):
                t0, ntl = sched["calls"][wvi][gg]
                if ntl == 0:
                    continue
                for sc in range(0, ntl, CHUNK):
                    k = min(CHUNK, ntl - sc)
                    m = msgs_p.tile(
                        [128, CHUNK * mrow], bf16, tag=f"msgs{gg}c{sc // CHUNK}"
                    )
                    msgs[(gg, sc // CHUNK)] = m
                    if ELEM128:
                        r, p = gg >> 1, gg & 1
                        _dma_gather_raw(
                            nc.gpsimd,
                            out_ap=m[:, : k * mrow].rearrange(
                                "p (t e) -> p t e", e=mrow
                            ),
                            in_ap=emb[
                                r * (HALF // 2) : (r + 1) * (HALF // 2),
                                p * D_FEAT : (p + 1) * D_FEAT,
                            ],
                            idxs_ap=idxg[gg][:, sc * 8 : (sc + k) * 8],
                            num_idxs=k * 128,
                            elem_size=D_FEAT,
                            elem_step=2 * D_FEAT,
                            queue_num=qn,
                        )
                    else:
                        nc.gpsimd.dma_gather(
                            out_ap=m[:, : k * mrow].rearrange(
                                "p (t e) -> p t e", e=mrow
                            ),
                            in_ap=emb[gg * GSZ : (gg + 1) * GSZ, :],
                            idxs_ap=idxg[gg][:, sc * 8 : (sc + k) * 8],
                            num_idxs=k * 128,
                            num_idxs_reg=k * 128,
                            elem_size=mrow,
                            queue_num=qn,
                        )
                    qn = (qn + 1) % N_QUEUES

            # ALL one-hots of the wave in one DVE op (2x_1p eligible):
            # ohX[p, d*ntw + t] = (iota[d] == dstloc[p, t])
            ohx = oh_p.tile([128, BUCKET * ntw_max], bf16)
            ohv = ohx[:, : BUCKET * ntw].rearrange("p (d t) -> p d t", t=ntw)
            nc.vector.tensor_tensor(
                out=ohv,
                in0=iota_rep[:].rearrange("p (d t) -> p d t", t=ntw_max)[
                    :, :, :ntw
                ],
                in1=dstw[:].unsqueeze(1).broadcast_to([128, BUCKET, ntw]),
                op=mybir.AluOpType.is_equal,
            )

            for bb in wv:
                passes = [
                    (gg, region_tile0[bb, gg] + j)
                    for gg in range(N_GROUPS)
                    for j in range(int(tbg[bb, gg]))
                ]
                psum = ps_p.tile([BUCKET, D_FEAT], f32)
                for i, (gg, t) in enumerate(passes):
                    tloc = int(t) - t0w
                    rel = int(t) - sched["calls"][wvi][gg][0]
                    off = (rel % CHUNK) * mrow
                    nc.tensor.matmul(
                        out=psum[:],
                        lhsT=ohv[:, :, tloc],
                        rhs=msgs[(gg, rel // CHUNK)][:, off : off + D_FEAT],
                        start=(i == 0),
                        stop=(i == len(passes) - 1),
                    )
                nrows = min(BUCKET, npc - bb * BUCKET)
                ot = outp_p.tile([BUCKET, D_FEAT], bf16)
                nc.scalar.activation(
                    out=ot[:],
                    in_=psum[:],
                    func=mybir.ActivationFunctionType.Copy,
                    scale=recall[:, bb : bb + 1],
                )
                nc.sync.dma_start(
                    out=out[bb * BUCKET : bb * BUCKET + nrows, :],
                    in_=ot[:nrows, :],
                )

    nc.compile()
    return nc


_CACHE = {}


def _run(embeddings, src, dst, trace=False, trace_kwargs=None):
    embeddings = np.asarray(embeddings, dtype=np.float32)
    if ELEM128:
        embp = np.zeros((N_PAD, D_FEAT), bfloat16)
        embp[:N_NODES] = embeddings.astype(bfloat16)
        embp = embp.reshape(HALF, 2 * D_FEAT)
    else:
        embp = np.zeros((N_PAD, 2 * D_FEAT), bfloat16)
        embp[:N_NODES, :D_FEAT] = embeddings.astype(bfloat16)
    idxtab, dst_t, rec_t, iota_rep, sched = _prep(src, dst)
    key = sched["tbg"].tobytes()
    if key not in _CACHE:
        _CACHE[key] = _build(sched)
    nc = _CACHE[key]

    in_maps = [
        {
            "emb": embp,
            "idx_t": idxtab[c],
            "dst_t": dst_t[c],
            "rec_t": rec_t[c],
            "iota_t": iota_rep,
        }
        for c in range(N_CORES)
    ]
    res = run_bass_kernel_spmd(
        nc,
        in_maps,
        core_ids=list(range(N_CORES)),
        trace=trace,
        **(trace_kwargs or {}),
    )
    out = np.concatenate(
        [res.results[c]["out"] for c in range(N_CORES)], axis=0
    ).astype(np.float32)
    return out, res


def kernel(embeddings, src, dst):
    out, _ = _run(embeddings, src, dst, trace=False)
    return out
